# revision 21
# baseline (speedup 1.0000x reference)
"""BENDR contrastive-loss kernel for Trainium2 (8 NeuronCores).

Reference computation (see problem): for each (b, t):
  logits[b*T+t, 0]   = cos(z[b,:,t], c[b,:,t+1]) / TEMP
  logits[b*T+t, 1+k] = cos(z[b,:,t], z[b,:,n(b,t,k)]) / TEMP
with n(b,t,k) = negative_inds[b, t*K+k] (row-local), TEMP=0.5.

Strategy: data-parallel over batch (2 rows per core). Every negative logit is
an entry of the row Gram matrix G = Zs^T Zs (Zs = sqrt(2)/||z_t|| scaled
columns), which is symmetric -- the device computes only the upper-triangular
128-row block stripes (53% of the full square) in fp8-e4m3 DoubleRow matmuls
(2x PE rate; operands pre-scaled by 64 so e4m3 stays in normal range, the
PSUM->fp16 drain descales by 1/4096). The positive logit needs only
diag(Zs^T Cs), computed without any per-block matmul: colsum(z*c) via a
ones-matmul, scaled by both reciprocal norms -- one fp16 row per batch row.

Norms: squares on ACT (Square activation, bf16 out), partition-reduction via
ones-matmul, sqrt on ACT reading PSUM, then the fast custom-DVE
reciprocal_approx_fast (the plain DVE reciprocal costs 3.2us per [128,512]
tile -- 5x slower). The z*c product and the positive-row scaling run on the
otherwise-idle GPSIMD engine.

The host does the final index-pick (pure indexing / unshard) with a
symmetric lookup: G[t,n] is stored at block min(t,n)//128.
"""

import sys

for _p in ("/opt/trn_rl_repo",):
    if _p not in sys.path:
        sys.path.append(_p)

import numpy as np

import concourse.bass as bass
import concourse.mybir as mybir
from concourse import tile as _tile
from concourse.tile import TileContext
from concourse.bass_utils import run_bass_kernel_spmd

dt = mybir.dt


B, F, T, K = 16, 256, 2048, 20
NCORES = 8
ROWS = B // NCORES          # batch rows per core
NBLK = T // 128             # t-blocks per batch row
FCH = F // 128              # f chunks (partition dim)
import os as _os
USE_FP8 = _os.environ.get("K_FP8", "1") == "1"  # fp8-e4m3 DoubleRow gram (2x PE)
USE_RECIP_FAST = _os.environ.get("K_RECIP_FAST", "1") == "1"
USE_GPSIMD = _os.environ.get("K_GPSIMD", "1") == "1"
USE_ZIGZAG = _os.environ.get("K_ZIGZAG", "1") == "1"
FP8_SCALE = 64.0            # operand pre-scale (e4m3 normal range)

# ---------------------------------------------------------------------------
# Walrus in this container rejects instructions that carry more than one
# semaphore wait ("Too many sync wait commands").  Two shims fix that: the
# tile tail drain gets its waits on single-wait NOPs, and a post-pass splits
# any remaining multi-wait instruction.
# ---------------------------------------------------------------------------


def _patched_drain_and_barrier(self, tick_clock, wait_clock):
    nop0 = self.nc.sync.nop(nofuse=True, hint="tail_wait")
    wait_clock.add_sem_waits(
        nop0.ins, _tile.ScopedClock({None: tick_clock.global_clock})
    )
    si = nop0.ins.sync_info
    if si is not None and len(si.on_wait) > 1:
        waits = list(si.on_wait)
        nop0.ins.sync_info = mybir.SyncInfo(
            on_wait=waits[:1], on_update=list(si.on_update)
        )
        for w in waits[1:]:
            nopi = self.nc.sync.nop(nofuse=True, hint="tail_wait")
            nopi.ins.sync_info = mybir.SyncInfo(on_wait=[w], on_update=[])
    self.nc.sync.drain()
    self.nc.all_engine_barrier()
    assert self.sems is not None
    popped = self.nc._tile_sem_poison_stack.pop()
    assert popped is self._sem_poison
    self.nc.clear_and_free_semaphores(list(self.sems.allocated().values()))
    self.nc.all_engine_barrier()


_tile.TileContext._drain_and_barrier = _patched_drain_and_barrier

_wnop_counter = [0]


def split_excess_waits(nc, cap=1):
    for f in nc.m.functions:
        for bb in f.blocks:
            insts = bb.instructions
            out = []
            changed = False
            for inst in list(insts):
                si = getattr(inst, "sync_info", None)
                waits = list(si.on_wait) if si is not None else []
                if len(waits) > cap:
                    keep = waits[-cap:]
                    for w in waits[: len(waits) - cap]:
                        _wnop_counter[0] += 1
                        nop = mybir.InstNoOp(
                            name=f"wnop-{_wnop_counter[0]}", ins=[], outs=[]
                        )
                        nop.engine = inst.engine
                        nop.sync_info = mybir.SyncInfo(on_wait=[w], on_update=[])
                        out.append(nop)
                    inst.sync_info = mybir.SyncInfo(
                        on_wait=keep, on_update=list(si.on_update)
                    )
                    changed = True
                out.append(inst)
            if changed:
                insts[:] = out


def dedup_ldweights(nc):
    """The tile lowering emits an explicit InstLdweights before every
    InstMatmult.  Consecutive matmuls that share the stationary operand
    (same AP + tile position) don't need the reload -- the PE keeps its
    weights.  Convert redundant loads into NoOps (keeping their sync info)."""
    n = 0
    for f in nc.m.functions:
        for bb in f.blocks:
            insts = bb.instructions
            last_key = None
            out = []
            changed = False
            for inst in list(insts):
                tn = type(inst).__name__
                if tn == "InstLdweights":
                    key = (
                        str(inst.ins[0]),
                        tuple(inst.tile_position or ()),
                        tuple(inst.tile_size or ()),
                        bool(inst.is_transpose),
                    )
                    if key == last_key:
                        nop = mybir.InstNoOp(name=f"ldwnop-{n}", ins=[], outs=[])
                        n += 1
                        nop.engine = inst.engine
                        si = inst.sync_info
                        if si is not None:
                            nop.sync_info = mybir.SyncInfo(
                                on_wait=list(si.on_wait), on_update=list(si.on_update)
                            )
                        out.append(nop)
                        changed = True
                        continue
                    last_key = key
                elif tn == "InstMatmult":
                    if inst.is_transpose:
                        last_key = None
                out.append(inst)
            if changed:
                insts[:] = out
    return n


# ---------------------------------------------------------------------------
# Device program
# ---------------------------------------------------------------------------


def _act_raw(nc, out, in_, func, scale=1.0):
    """Raw InstActivation emitter: bass blocks Rsqrt/Reciprocal on ACT for
    accuracy reasons, but measured on this hardware Rsqrt is accurate to
    4.4e-5 max rel err -- far below this problem's tolerance."""
    eng = nc.scalar
    bias = nc.const_aps.scalar_like(0.0, in_)
    inputs = [eng.lower_ap(in_), eng.lower_ap(bias)]
    for arg in [scale, 0.0]:
        inputs.append(mybir.ImmediateValue(dtype=mybir.dt.float32, value=float(arg)))
    return eng.add_instruction(
        mybir.InstActivation(
            name=nc.get_next_instruction_name(),
            func=func,
            ins=inputs,
            outs=[eng.lower_ap(out)],
        )
    )


def build_program():
    op_dt = dt.float8e4 if USE_FP8 else dt.bfloat16
    descale = 1.0 / (FP8_SCALE * FP8_SCALE) if USE_FP8 else 1.0
    z_sqrt_scale = 1.0 / (2.0 * FP8_SCALE * FP8_SCALE) if USE_FP8 else 0.5
    c_sqrt_scale = (FP8_SCALE * FP8_SCALE) / 2.0 if USE_FP8 else 0.5

    nc = bass.Bass("TRN2", num_devices=NCORES)
    z_in = nc.dram_tensor("z", [ROWS, F, T], dt.float32, kind="ExternalInput")
    c_in = nc.dram_tensor("c", [ROWS, F, T], dt.float32, kind="ExternalInput")
    sims_out = nc.dram_tensor(
        "sims", [ROWS, NBLK, 128, T], dt.float16, kind="ExternalOutput"
    )
    pos_out = nc.dram_tensor("pos", [ROWS, T], dt.float16, kind="ExternalOutput")

    with TileContext(nc) as tc:
        with (
            tc.tile_pool(name="io", bufs=2) as io_pool,
            tc.tile_pool(name="sq", bufs=1) as sq_pool,
            tc.tile_pool(name="rn", bufs=2) as rn_pool,
            tc.tile_pool(name="tmp", bufs=2) as tmp_pool,
            tc.tile_pool(name="scaled", bufs=2) as scaled,
            tc.tile_pool(name="outp", bufs=8) as outp,
            tc.tile_pool(name="gram_ps", bufs=5, space="PSUM") as gram_ps,
            tc.tile_pool(name="stat_ps", bufs=1, space="PSUM") as stat_ps,
        ):
            ones16 = io_pool.tile([128, 128], dt.bfloat16, name="ones16")
            nc.vector.memset(ones16[:], 1.0)

            copy_cols = {"act": 26000.0, "dve": 15000.0}
            COPY_W = {"act": 0.91, "dve": 1.15}

            def drain_psum(dst, src, wc):
                if copy_cols["act"] + COPY_W["act"] * wc <= copy_cols["dve"] + COPY_W["dve"] * wc:
                    eng = "act"
                else:
                    eng = "dve"
                copy_cols[eng] += COPY_W[eng] * wc
                if eng == "act":
                    nc.scalar.activation(
                        dst, src, mybir.ActivationFunctionType.Copy, scale=descale
                    )
                else:
                    nc.vector.tensor_scalar(
                        dst, src, descale, None, mybir.AluOpType.mult
                    )

            state = {}

            def stats_load(r, pieces):
                st = state[r] = {}
                zf = io_pool.tile([128, FCH, T], dt.float32, name="zf", tag="zf")
                cf = io_pool.tile([128, FCH, T], dt.float32, name="cf", tag="cf")
                sqz = sq_pool.tile([128, FCH, T], dt.bfloat16, name="sqz", tag="sqz")
                sqc = sq_pool.tile([128, FCH, T], dt.bfloat16, name="sqc", tag="sqc")
                zc = sq_pool.tile([128, FCH, T], dt.bfloat16, name="zc", tag="zc")
                st.update(zf=zf, cf=cf, sqz=sqz, sqc=sqc, zc=zc)
                pw = T // pieces
                for p in range(pieces):
                    sl = slice(pw * p, pw * (p + 1))
                    for j in range(FCH):
                        nc.sync.dma_start(
                            out=zf[:, j, sl],
                            in_=z_in[r, 128 * j : 128 * (j + 1), sl],
                        )
                        nc.sync.dma_start(
                            out=cf[:, j, sl],
                            in_=c_in[r, 128 * j : 128 * (j + 1), sl],
                        )
                for p in range(pieces):
                    sl = slice(pw * p, pw * (p + 1))
                    for j in range(FCH):
                        nc.scalar.activation(
                            sqz[:, j, sl], zf[:, j, sl],
                            mybir.ActivationFunctionType.Square,
                        )
                        nc.scalar.activation(
                            sqc[:, j, sl], cf[:, j, sl],
                            mybir.ActivationFunctionType.Square,
                        )
                        if USE_GPSIMD:
                            nc.gpsimd.tensor_mul(zc[:, j, sl], zf[:, j, sl], cf[:, j, sl])
                        else:
                            nc.vector.tensor_tensor(
                                out=zc[:, j, sl], in0=zf[:, j, sl],
                                in1=cf[:, j, sl], op=mybir.AluOpType.mult,
                            )
                st["rnz"] = rn_pool.tile([128, T], dt.float32, name="rnz", tag="rnz")
                st["rnc"] = rn_pool.tile([128, T], dt.float32, name="rnc", tag="rnc")
                st["pos16"] = rn_pool.tile([128, T], dt.float16, name="pos16", tag="pos16")

            def stats_norm(r, cc):
                st = state[r]
                sl = slice(512 * cc, 512 * (cc + 1))
                nz_ps = stat_ps.tile([128, 512], dt.float32, name="nz_ps", tag="ps_a")
                ncc_ps = stat_ps.tile([128, 512], dt.float32, name="ncc_ps", tag="ps_b")
                zc_ps = stat_ps.tile([128, 512], dt.float32, name="zc_ps", tag="ps_c")
                for j in range(FCH):
                    b, e = (j == 0), (j == FCH - 1)
                    nc.tensor.matmul(nz_ps[:], ones16[:], st["sqz"][:, j, sl], start=b, stop=e)
                    nc.tensor.matmul(ncc_ps[:], ones16[:], st["sqc"][:, j, sl], start=b, stop=e)
                    nc.tensor.matmul(zc_ps[:], ones16[:], st["zc"][:, j, sl], start=b, stop=e)
                pt = tmp_pool.tile([128, 512], dt.float32, name="pt", tag="pt")
                _act_raw(nc, st["rnz"][:, sl], nz_ps[:],
                         mybir.ActivationFunctionType.Rsqrt, scale=z_sqrt_scale)
                _act_raw(nc, st["rnc"][:, sl], ncc_ps[:],
                         mybir.ActivationFunctionType.Rsqrt, scale=c_sqrt_scale)
                nc.vector.tensor_tensor(
                    out=pt[:], in0=zc_ps[:], in1=st["rnz"][:, sl],
                    op=mybir.AluOpType.mult,
                )
                if USE_GPSIMD:
                    nc.gpsimd.tensor_mul(st["pos16"][:, sl], pt[:], st["rnc"][:, sl])
                else:
                    nc.vector.tensor_tensor(
                        out=st["pos16"][:, sl], in0=pt[:], in1=st["rnc"][:, sl],
                        op=mybir.AluOpType.mult,
                    )

            def stats_zs(r, j, cc=None):
                st = state[r]
                if "zs" not in st:
                    st["zs"] = scaled.tile([128, FCH, T], op_dt, name="zs", tag="zs")
                sl = slice(0, T) if cc is None else slice(512 * cc, 512 * (cc + 1))
                nc.vector.tensor_tensor(
                    out=st["zs"][:, j, sl], in0=st["zf"][:, j, sl],
                    in1=st["rnz"][:, sl], op=mybir.AluOpType.mult,
                )

            def stats_posdma(r):
                nc.sync.dma_start(out=pos_out[r, :], in_=state[r]["pos16"][0:1, :])

            def emit_gram_chunk(r, tau, off, wc, odst):
                zs = state[r]["zs"]
                t0 = 128 * tau
                ps = gram_ps.tile([128, 512], dt.float32, name="ps", tag="ps")
                if USE_FP8:
                    nc.tensor.matmul(
                        ps[:, :wc], zs[:, :, t0 : t0 + 128],
                        zs[:, :, t0 + off : t0 + off + wc],
                        start=True, stop=True,
                        perf_mode=mybir.MatmulPerfMode.DoubleRow,
                    )
                else:
                    for j in range(FCH):
                        nc.tensor.matmul(
                            ps[:, :wc], zs[:, j, t0 : t0 + 128],
                            zs[:, j, t0 + off : t0 + off + wc],
                            start=(j == 0), stop=(j == FCH - 1),
                        )
                drain_psum(odst[:, off : off + wc], ps[:, :wc], wc)

            def emit_gram_block(r, tau):
                w = T - 128 * tau
                otile = outp.tile([128, T], dt.float16, name="otile", tag="otile")
                off = 0
                while off < w:
                    wc = min(512, w - off)
                    emit_gram_chunk(r, tau, off, wc, otile)
                    off += wc
                nc.sync.dma_start(out=sims_out[r, tau, :, :w], in_=otile[:, :w])

            sid = nc.enter_named_scope("stats_r0", False)[0]
            stats_load(0, pieces=4)
            for cc in range(4):
                stats_norm(0, cc)
                for j in range(FCH):
                    stats_zs(0, j, cc)
            stats_posdma(0)
            nc.leave_named_scope("stats_r0", sid, False)

            for r in range(ROWS):
                nxt = r + 1
                phases = {}
                if nxt < ROWS:
                    phases = {
                        1: lambda: stats_load(nxt, pieces=1),
                        3: lambda: stats_norm(nxt, 0),
                        4: lambda: stats_norm(nxt, 1),
                        5: lambda: stats_norm(nxt, 2),
                        6: lambda: stats_norm(nxt, 3),
                        7: lambda: stats_zs(nxt, 0),
                        8: lambda: stats_zs(nxt, 1),
                        9: lambda: stats_posdma(nxt),
                    }
                sid = nc.enter_named_scope(f"gram_r{r}", False)[0]
                for tau in range(NBLK):
                    if tau in phases:
                        phases[tau]()
                    emit_gram_block(r, tau)
                nc.leave_named_scope(f"gram_r{r}", sid, False)

    dedup_ldweights(nc)
    split_excess_waits(nc)
    return nc


_PROGRAM = None


def _get_program():
    global _PROGRAM
    if _PROGRAM is None:
        _PROGRAM = build_program()
    return _PROGRAM


def kernel(z, c, negative_inds, _trace=False):
    z = np.ascontiguousarray(np.asarray(z, dtype=np.float32))
    c = np.ascontiguousarray(np.asarray(c, dtype=np.float32))
    ni = np.asarray(negative_inds)
    assert z.shape == (B, F, T) and c.shape == (B, F, T + 1)

    c_sl = np.ascontiguousarray(c[:, :, 1:])  # [B, F, T]

    nc = _get_program()
    in_maps = []
    for core in range(NCORES):
        rs = slice(core * ROWS, (core + 1) * ROWS)
        in_maps.append({"z": z[rs], "c": c_sl[rs]})

    res = run_bass_kernel_spmd(nc, in_maps, list(range(NCORES)), trace=_trace)

    sims = np.concatenate(
        [res.results[i]["sims"] for i in range(NCORES)], axis=0
    )
    pos = np.concatenate(
        [res.results[i]["pos"] for i in range(NCORES)], axis=0
    )

    n = ni.reshape(B, T, K).astype(np.int64)
    t = np.arange(T, dtype=np.int64)[None, :, None]
    swap = (n >> 7) < (t >> 7)
    a = np.where(swap, n, t)
    bcol = np.where(swap, t, n)
    bidx = np.arange(B)[:, None, None]
    neg = sims[bidx, a >> 7, a & 127, bcol - ((a >> 7) << 7)]
    logits = np.concatenate([pos[:, :, None], neg], axis=2).astype(np.float32)
    out = logits.reshape(B * T, K + 1)
    if _trace:
        return out, res
    return out


if __name__ == "__main__":
    rng = np.random.default_rng(0)
    z = rng.standard_normal((B, F, T), dtype=np.float32)
    c = rng.standard_normal((B, F, T + 1), dtype=np.float32)
    ni = rng.integers(0, T - 1, size=(B, T * K)).astype(np.int64)
    out = kernel(z=z, c=c, negative_inds=ni)
    print("out", out.shape, out.dtype, np.isfinite(out).all())


# revision 22
# speedup vs baseline: 1.1834x; 1.1834x over previous
"""BENDR contrastive-loss kernel for Trainium2 (8 NeuronCores).

Reference computation (see problem): for each (b, t):
  logits[b*T+t, 0]   = cos(z[b,:,t], c[b,:,t+1]) / TEMP
  logits[b*T+t, 1+k] = cos(z[b,:,t], z[b,:,n(b,t,k)]) / TEMP
with n(b,t,k) = negative_inds[b, t*K+k] (row-local), TEMP=0.5.

Strategy: data-parallel over batch (2 rows per core). Every negative logit is
an entry of the row Gram matrix G = Zs^T Zs (Zs = sqrt(2)/||z_t|| scaled
columns), which is symmetric -- the device computes only the upper-triangular
128-row block stripes (53% of the full square) in fp8-e4m3 DoubleRow matmuls
(2x PE rate; operands pre-scaled by 64 so e4m3 stays in normal range, the
PSUM->fp16 drain descales by 1/4096). The positive logit needs only
diag(Zs^T Cs), computed without any per-block matmul: colsum(z*c) via a
ones-matmul, scaled by both reciprocal norms -- one fp16 row per batch row.

Norms: squares on ACT (Square activation, bf16 out), partition-reduction via
ones-matmul, sqrt on ACT reading PSUM, then the fast custom-DVE
reciprocal_approx_fast (the plain DVE reciprocal costs 3.2us per [128,512]
tile -- 5x slower). The z*c product and the positive-row scaling run on the
otherwise-idle GPSIMD engine.

The host does the final index-pick (pure indexing / unshard) with a
symmetric lookup: G[t,n] is stored at block min(t,n)//128.
"""

import sys

for _p in ("/opt/trn_rl_repo",):
    if _p not in sys.path:
        sys.path.append(_p)

import numpy as np

import concourse.bass as bass
import concourse.mybir as mybir
from concourse import tile as _tile
from concourse.tile import TileContext
from concourse.bass_utils import run_bass_kernel_spmd

dt = mybir.dt


B, F, T, K = 16, 256, 2048, 20
NCORES = 8
ROWS = B // NCORES          # batch rows per core
NBLK = T // 128             # t-blocks per batch row
FCH = F // 128              # f chunks (partition dim)
import os as _os
USE_FP8 = _os.environ.get("K_FP8", "1") == "1"  # fp8-e4m3 DoubleRow gram (2x PE)
USE_RECIP_FAST = _os.environ.get("K_RECIP_FAST", "1") == "1"
USE_GPSIMD = _os.environ.get("K_GPSIMD", "1") == "1"
USE_ZIGZAG = _os.environ.get("K_ZIGZAG", "1") == "1"
FP8_SCALE = 64.0            # operand pre-scale (e4m3 normal range)

# ---------------------------------------------------------------------------
# Walrus in this container rejects instructions that carry more than one
# semaphore wait ("Too many sync wait commands").  Two shims fix that: the
# tile tail drain gets its waits on single-wait NOPs, and a post-pass splits
# any remaining multi-wait instruction.
# ---------------------------------------------------------------------------


def _patched_drain_and_barrier(self, tick_clock, wait_clock):
    nop0 = self.nc.sync.nop(nofuse=True, hint="tail_wait")
    wait_clock.add_sem_waits(
        nop0.ins, _tile.ScopedClock({None: tick_clock.global_clock})
    )
    si = nop0.ins.sync_info
    if si is not None and len(si.on_wait) > 1:
        waits = list(si.on_wait)
        nop0.ins.sync_info = mybir.SyncInfo(
            on_wait=waits[:1], on_update=list(si.on_update)
        )
        for w in waits[1:]:
            nopi = self.nc.sync.nop(nofuse=True, hint="tail_wait")
            nopi.ins.sync_info = mybir.SyncInfo(on_wait=[w], on_update=[])
    self.nc.sync.drain()
    self.nc.all_engine_barrier()
    assert self.sems is not None
    popped = self.nc._tile_sem_poison_stack.pop()
    assert popped is self._sem_poison
    self.nc.clear_and_free_semaphores(list(self.sems.allocated().values()))
    self.nc.all_engine_barrier()


_tile.TileContext._drain_and_barrier = _patched_drain_and_barrier

_wnop_counter = [0]


def split_excess_waits(nc, cap=1):
    for f in nc.m.functions:
        for bb in f.blocks:
            insts = bb.instructions
            out = []
            changed = False
            for inst in list(insts):
                si = getattr(inst, "sync_info", None)
                waits = list(si.on_wait) if si is not None else []
                if len(waits) > cap:
                    keep = waits[-cap:]
                    for w in waits[: len(waits) - cap]:
                        _wnop_counter[0] += 1
                        nop = mybir.InstNoOp(
                            name=f"wnop-{_wnop_counter[0]}", ins=[], outs=[]
                        )
                        nop.engine = inst.engine
                        nop.sync_info = mybir.SyncInfo(on_wait=[w], on_update=[])
                        out.append(nop)
                    inst.sync_info = mybir.SyncInfo(
                        on_wait=keep, on_update=list(si.on_update)
                    )
                    changed = True
                out.append(inst)
            if changed:
                insts[:] = out


def dedup_ldweights(nc):
    """The tile lowering emits an explicit InstLdweights before every
    InstMatmult.  Consecutive matmuls that share the stationary operand
    (same AP + tile position) don't need the reload -- the PE keeps its
    weights.  Convert redundant loads into NoOps (keeping their sync info)."""
    n = 0
    for f in nc.m.functions:
        for bb in f.blocks:
            insts = bb.instructions
            last_key = None
            out = []
            changed = False
            for inst in list(insts):
                tn = type(inst).__name__
                if tn == "InstLdweights":
                    key = (
                        str(inst.ins[0]),
                        tuple(inst.tile_position or ()),
                        tuple(inst.tile_size or ()),
                        bool(inst.is_transpose),
                    )
                    if key == last_key:
                        nop = mybir.InstNoOp(name=f"ldwnop-{n}", ins=[], outs=[])
                        n += 1
                        nop.engine = inst.engine
                        si = inst.sync_info
                        if si is not None:
                            nop.sync_info = mybir.SyncInfo(
                                on_wait=list(si.on_wait), on_update=list(si.on_update)
                            )
                        out.append(nop)
                        changed = True
                        continue
                    last_key = key
                elif tn == "InstMatmult":
                    if inst.is_transpose:
                        last_key = None
                out.append(inst)
            if changed:
                insts[:] = out
    return n


# ---------------------------------------------------------------------------
# Device program
# ---------------------------------------------------------------------------


def _act_raw(nc, out, in_, func, scale=1.0):
    """Raw InstActivation emitter: bass blocks Rsqrt/Reciprocal on ACT for
    accuracy reasons, but measured on this hardware Rsqrt is accurate to
    4.4e-5 max rel err -- far below this problem's tolerance."""
    eng = nc.scalar
    bias = nc.const_aps.scalar_like(0.0, in_)
    inputs = [eng.lower_ap(in_), eng.lower_ap(bias)]
    for arg in [scale, 0.0]:
        inputs.append(mybir.ImmediateValue(dtype=mybir.dt.float32, value=float(arg)))
    return eng.add_instruction(
        mybir.InstActivation(
            name=nc.get_next_instruction_name(),
            func=func,
            ins=inputs,
            outs=[eng.lower_ap(out)],
        )
    )


def build_program():
    op_dt = dt.float8e4 if USE_FP8 else dt.bfloat16
    descale = 1.0 / (FP8_SCALE * FP8_SCALE) if USE_FP8 else 1.0
    z_sqrt_scale = 1.0 / (2.0 * FP8_SCALE * FP8_SCALE) if USE_FP8 else 0.5
    c_sqrt_scale = (FP8_SCALE * FP8_SCALE) / 2.0 if USE_FP8 else 0.5

    nc = bass.Bass("TRN2", num_devices=NCORES)
    z_in = nc.dram_tensor("z", [ROWS, F, T], dt.float32, kind="ExternalInput")
    c_in = nc.dram_tensor("c", [ROWS, F, T], dt.float32, kind="ExternalInput")
    sims_out = nc.dram_tensor(
        "sims", [ROWS, NBLK, 128, T], dt.float16, kind="ExternalOutput"
    )
    pos_out = nc.dram_tensor("pos", [ROWS, T], dt.float16, kind="ExternalOutput")

    with TileContext(nc) as tc:
        with (
            tc.tile_pool(name="io", bufs=2) as io_pool,
            tc.tile_pool(name="sq", bufs=1) as sq_pool,
            tc.tile_pool(name="rn", bufs=2) as rn_pool,
            tc.tile_pool(name="tmp", bufs=2) as tmp_pool,
            tc.tile_pool(name="scaled", bufs=2) as scaled,
            tc.tile_pool(name="outp", bufs=6) as outp,
            tc.tile_pool(name="gram_ps", bufs=5, space="PSUM") as gram_ps,
            tc.tile_pool(name="stat_ps", bufs=1, space="PSUM") as stat_ps,
        ):
            ones16 = io_pool.tile([128, 128], dt.bfloat16, name="ones16")
            nc.vector.memset(ones16[:], 1.0)

            copy_cols = {"act": 26000.0, "dve": 15000.0}
            COPY_W = {"act": 0.91, "dve": 1.15}

            def drain_psum(dst, src, wc):
                if copy_cols["act"] + COPY_W["act"] * wc <= copy_cols["dve"] + COPY_W["dve"] * wc:
                    eng = "act"
                else:
                    eng = "dve"
                copy_cols[eng] += COPY_W[eng] * wc
                if eng == "act":
                    nc.scalar.activation(
                        dst, src, mybir.ActivationFunctionType.Copy, scale=descale
                    )
                else:
                    nc.vector.tensor_scalar(
                        dst, src, descale, None, mybir.AluOpType.mult
                    )

            state = {}

            def stats_load(r, pieces):
                st = state[r] = {}
                zf = io_pool.tile([128, FCH, T], dt.float32, name="zf", tag="zf")
                cf = io_pool.tile([128, FCH, T], dt.float32, name="cf", tag="cf")
                sqz = sq_pool.tile([128, FCH, T], dt.bfloat16, name="sqz", tag="sqz")
                sqc = sq_pool.tile([128, FCH, T], dt.bfloat16, name="sqc", tag="sqc")
                zc = sq_pool.tile([128, FCH, T], dt.bfloat16, name="zc", tag="zc")
                st.update(zf=zf, cf=cf, sqz=sqz, sqc=sqc, zc=zc)
                pw = T // pieces
                for p in range(pieces):
                    sl = slice(pw * p, pw * (p + 1))
                    for j in range(FCH):
                        nc.sync.dma_start(
                            out=zf[:, j, sl],
                            in_=z_in[r, 128 * j : 128 * (j + 1), sl],
                        )
                        nc.sync.dma_start(
                            out=cf[:, j, sl],
                            in_=c_in[r, 128 * j : 128 * (j + 1), sl],
                        )
                for p in range(pieces):
                    sl = slice(pw * p, pw * (p + 1))
                    for j in range(FCH):
                        nc.scalar.activation(
                            sqz[:, j, sl], zf[:, j, sl],
                            mybir.ActivationFunctionType.Square,
                        )
                        nc.scalar.activation(
                            sqc[:, j, sl], cf[:, j, sl],
                            mybir.ActivationFunctionType.Square,
                        )
                        if USE_GPSIMD:
                            nc.gpsimd.tensor_mul(zc[:, j, sl], zf[:, j, sl], cf[:, j, sl])
                        else:
                            nc.vector.tensor_tensor(
                                out=zc[:, j, sl], in0=zf[:, j, sl],
                                in1=cf[:, j, sl], op=mybir.AluOpType.mult,
                            )
                st["rnz"] = rn_pool.tile([128, T], dt.float32, name="rnz", tag="rnz")
                st["rnc"] = rn_pool.tile([128, T], dt.float32, name="rnc", tag="rnc")
                st["pos16"] = rn_pool.tile([128, T], dt.float16, name="pos16", tag="pos16")

            def stats_norm(r, cc):
                st = state[r]
                sl = slice(512 * cc, 512 * (cc + 1))
                nz_ps = stat_ps.tile([128, 512], dt.float32, name="nz_ps", tag="ps_a")
                ncc_ps = stat_ps.tile([128, 512], dt.float32, name="ncc_ps", tag="ps_b")
                zc_ps = stat_ps.tile([128, 512], dt.float32, name="zc_ps", tag="ps_c")
                for j in range(FCH):
                    b, e = (j == 0), (j == FCH - 1)
                    nc.tensor.matmul(nz_ps[:], ones16[:], st["sqz"][:, j, sl], start=b, stop=e)
                    nc.tensor.matmul(ncc_ps[:], ones16[:], st["sqc"][:, j, sl], start=b, stop=e)
                    nc.tensor.matmul(zc_ps[:], ones16[:], st["zc"][:, j, sl], start=b, stop=e)
                pt = tmp_pool.tile([128, 512], dt.float32, name="pt", tag="pt")
                _act_raw(nc, st["rnz"][:, sl], nz_ps[:],
                         mybir.ActivationFunctionType.Rsqrt, scale=z_sqrt_scale)
                _act_raw(nc, st["rnc"][:, sl], ncc_ps[:],
                         mybir.ActivationFunctionType.Rsqrt, scale=c_sqrt_scale)
                nc.vector.tensor_tensor(
                    out=pt[:], in0=zc_ps[:], in1=st["rnz"][:, sl],
                    op=mybir.AluOpType.mult,
                )
                if USE_GPSIMD:
                    nc.gpsimd.tensor_mul(st["pos16"][:, sl], pt[:], st["rnc"][:, sl])
                else:
                    nc.vector.tensor_tensor(
                        out=st["pos16"][:, sl], in0=pt[:], in1=st["rnc"][:, sl],
                        op=mybir.AluOpType.mult,
                    )

            def stats_zs(r, j, cc=None):
                st = state[r]
                if "zs" not in st:
                    st["zs"] = scaled.tile([128, FCH, T], op_dt, name="zs", tag="zs")
                sl = slice(0, T) if cc is None else slice(512 * cc, 512 * (cc + 1))
                nc.vector.tensor_tensor(
                    out=st["zs"][:, j, sl], in0=st["zf"][:, j, sl],
                    in1=st["rnz"][:, sl], op=mybir.AluOpType.mult,
                )

            def stats_posdma(r):
                nc.sync.dma_start(out=pos_out[r, :], in_=state[r]["pos16"][0:1, :])

            def emit_gram_chunk(r, tau, off, wc, odst):
                zs = state[r]["zs"]
                t0 = 128 * tau
                ps = gram_ps.tile([128, 512], dt.float32, name="ps", tag="ps")
                if USE_FP8:
                    nc.tensor.matmul(
                        ps[:, :wc], zs[:, :, t0 : t0 + 128],
                        zs[:, :, t0 + off : t0 + off + wc],
                        start=True, stop=True,
                        perf_mode=mybir.MatmulPerfMode.DoubleRow,
                    )
                else:
                    for j in range(FCH):
                        nc.tensor.matmul(
                            ps[:, :wc], zs[:, j, t0 : t0 + 128],
                            zs[:, j, t0 + off : t0 + off + wc],
                            start=(j == 0), stop=(j == FCH - 1),
                        )
                drain_psum(odst[:, off : off + wc], ps[:, :wc], wc)

            def emit_gram_block(r, tau):
                w = T - 128 * tau
                otile = outp.tile([128, T], dt.float16, name="otile", tag="otile")
                off = 0
                while off < w:
                    wc = min(512, w - off)
                    emit_gram_chunk(r, tau, off, wc, otile)
                    off += wc
                nc.sync.dma_start(out=sims_out[r, tau, :, :w], in_=otile[:, :w])

            sid = nc.enter_named_scope("stats_r0", False)[0]
            stats_load(0, pieces=4)
            for cc in range(4):
                stats_norm(0, cc)
                for j in range(FCH):
                    stats_zs(0, j, cc)
            stats_posdma(0)
            nc.leave_named_scope("stats_r0", sid, False)

            for r in range(ROWS):
                nxt = r + 1
                phases = {}
                if nxt < ROWS:
                    phases = {
                        1: lambda: stats_load(nxt, pieces=1),
                        3: lambda: stats_norm(nxt, 0),
                        4: lambda: stats_norm(nxt, 1),
                        5: lambda: stats_norm(nxt, 2),
                        6: lambda: stats_norm(nxt, 3),
                        7: lambda: stats_zs(nxt, 0),
                        8: lambda: stats_zs(nxt, 1),
                        9: lambda: stats_posdma(nxt),
                    }
                sid = nc.enter_named_scope(f"gram_r{r}", False)[0]
                for tau in range(NBLK):
                    if tau in phases:
                        phases[tau]()
                    emit_gram_block(r, tau)
                nc.leave_named_scope(f"gram_r{r}", sid, False)

    dedup_ldweights(nc)
    split_excess_waits(nc)
    return nc


_PROGRAM = None


def _get_program():
    global _PROGRAM
    if _PROGRAM is None:
        _PROGRAM = build_program()
    return _PROGRAM


def kernel(z, c, negative_inds, _trace=False):
    z = np.ascontiguousarray(np.asarray(z, dtype=np.float32))
    c = np.ascontiguousarray(np.asarray(c, dtype=np.float32))
    ni = np.asarray(negative_inds)
    assert z.shape == (B, F, T) and c.shape == (B, F, T + 1)

    c_sl = np.ascontiguousarray(c[:, :, 1:])  # [B, F, T]

    nc = _get_program()
    in_maps = []
    for core in range(NCORES):
        rs = slice(core * ROWS, (core + 1) * ROWS)
        in_maps.append({"z": z[rs], "c": c_sl[rs]})

    res = run_bass_kernel_spmd(nc, in_maps, list(range(NCORES)), trace=_trace)

    sims = np.concatenate(
        [res.results[i]["sims"] for i in range(NCORES)], axis=0
    )
    pos = np.concatenate(
        [res.results[i]["pos"] for i in range(NCORES)], axis=0
    )

    n = ni.reshape(B, T, K).astype(np.int64)
    t = np.arange(T, dtype=np.int64)[None, :, None]
    swap = (n >> 7) < (t >> 7)
    a = np.where(swap, n, t)
    bcol = np.where(swap, t, n)
    bidx = np.arange(B)[:, None, None]
    neg = sims[bidx, a >> 7, a & 127, bcol - ((a >> 7) << 7)]
    logits = np.concatenate([pos[:, :, None], neg], axis=2).astype(np.float32)
    out = logits.reshape(B * T, K + 1)
    if _trace:
        return out, res
    return out


if __name__ == "__main__":
    rng = np.random.default_rng(0)
    z = rng.standard_normal((B, F, T), dtype=np.float32)
    c = rng.standard_normal((B, F, T + 1), dtype=np.float32)
    ni = rng.integers(0, T - 1, size=(B, T * K)).astype(np.int64)
    out = kernel(z=z, c=c, negative_inds=ni)
    print("out", out.shape, out.dtype, np.isfinite(out).all())


# revision 23
# speedup vs baseline: 1.2059x; 1.0190x over previous
"""BENDR contrastive-loss kernel for Trainium2 (8 NeuronCores).

Reference computation (see problem): for each (b, t):
  logits[b*T+t, 0]   = cos(z[b,:,t], c[b,:,t+1]) / TEMP
  logits[b*T+t, 1+k] = cos(z[b,:,t], z[b,:,n(b,t,k)]) / TEMP
with n(b,t,k) = negative_inds[b, t*K+k] (row-local), TEMP=0.5.

Strategy: data-parallel over batch (2 rows per core). Every negative logit is
an entry of the row Gram matrix G = Zs^T Zs (Zs = sqrt(2)/||z_t|| scaled
columns), which is symmetric -- the device computes only the upper-triangular
128-row block stripes (53% of the full square) in fp8-e4m3 DoubleRow matmuls
(2x PE rate; operands pre-scaled by 64 so e4m3 stays in normal range, the
PSUM->fp16 drain descales by 1/4096). The positive logit needs only
diag(Zs^T Cs), computed without any per-block matmul: colsum(z*c) via a
ones-matmul, scaled by both reciprocal norms -- one fp16 row per batch row.

Norms: squares on ACT (Square activation, bf16 out), partition-reduction via
ones-matmul, sqrt on ACT reading PSUM, then the fast custom-DVE
reciprocal_approx_fast (the plain DVE reciprocal costs 3.2us per [128,512]
tile -- 5x slower). The z*c product and the positive-row scaling run on the
otherwise-idle GPSIMD engine.

The host does the final index-pick (pure indexing / unshard) with a
symmetric lookup: G[t,n] is stored at block min(t,n)//128.
"""

import sys

for _p in ("/opt/trn_rl_repo",):
    if _p not in sys.path:
        sys.path.append(_p)

import numpy as np

import concourse.bass as bass
import concourse.mybir as mybir
from concourse import tile as _tile
from concourse.tile import TileContext
from concourse.bass_utils import run_bass_kernel_spmd

dt = mybir.dt


B, F, T, K = 16, 256, 2048, 20
NCORES = 8
ROWS = B // NCORES          # batch rows per core
NBLK = T // 128             # t-blocks per batch row
FCH = F // 128              # f chunks (partition dim)
import os as _os
USE_FP8 = _os.environ.get("K_FP8", "1") == "1"  # fp8-e4m3 DoubleRow gram (2x PE)
USE_RECIP_FAST = _os.environ.get("K_RECIP_FAST", "1") == "1"
USE_GPSIMD = _os.environ.get("K_GPSIMD", "1") == "1"
USE_ZIGZAG = _os.environ.get("K_ZIGZAG", "1") == "1"
FP8_SCALE = 64.0            # operand pre-scale (e4m3 normal range)

# ---------------------------------------------------------------------------
# Walrus in this container rejects instructions that carry more than one
# semaphore wait ("Too many sync wait commands").  Two shims fix that: the
# tile tail drain gets its waits on single-wait NOPs, and a post-pass splits
# any remaining multi-wait instruction.
# ---------------------------------------------------------------------------


def _patched_drain_and_barrier(self, tick_clock, wait_clock):
    nop0 = self.nc.sync.nop(nofuse=True, hint="tail_wait")
    wait_clock.add_sem_waits(
        nop0.ins, _tile.ScopedClock({None: tick_clock.global_clock})
    )
    si = nop0.ins.sync_info
    if si is not None and len(si.on_wait) > 1:
        waits = list(si.on_wait)
        nop0.ins.sync_info = mybir.SyncInfo(
            on_wait=waits[:1], on_update=list(si.on_update)
        )
        for w in waits[1:]:
            nopi = self.nc.sync.nop(nofuse=True, hint="tail_wait")
            nopi.ins.sync_info = mybir.SyncInfo(on_wait=[w], on_update=[])
    self.nc.sync.drain()
    self.nc.all_engine_barrier()
    assert self.sems is not None
    popped = self.nc._tile_sem_poison_stack.pop()
    assert popped is self._sem_poison
    self.nc.clear_and_free_semaphores(list(self.sems.allocated().values()))
    self.nc.all_engine_barrier()


_tile.TileContext._drain_and_barrier = _patched_drain_and_barrier

_wnop_counter = [0]


def split_excess_waits(nc, cap=1):
    for f in nc.m.functions:
        for bb in f.blocks:
            insts = bb.instructions
            out = []
            changed = False
            for inst in list(insts):
                si = getattr(inst, "sync_info", None)
                waits = list(si.on_wait) if si is not None else []
                if len(waits) > cap:
                    keep = waits[-cap:]
                    for w in waits[: len(waits) - cap]:
                        _wnop_counter[0] += 1
                        nop = mybir.InstNoOp(
                            name=f"wnop-{_wnop_counter[0]}", ins=[], outs=[]
                        )
                        nop.engine = inst.engine
                        nop.sync_info = mybir.SyncInfo(on_wait=[w], on_update=[])
                        out.append(nop)
                    inst.sync_info = mybir.SyncInfo(
                        on_wait=keep, on_update=list(si.on_update)
                    )
                    changed = True
                out.append(inst)
            if changed:
                insts[:] = out


def dedup_ldweights(nc):
    """The tile lowering emits an explicit InstLdweights before every
    InstMatmult.  Consecutive matmuls that share the stationary operand
    (same AP + tile position) don't need the reload -- the PE keeps its
    weights.  Convert redundant loads into NoOps (keeping their sync info)."""
    n = 0
    for f in nc.m.functions:
        for bb in f.blocks:
            insts = bb.instructions
            last_key = None
            out = []
            changed = False
            for inst in list(insts):
                tn = type(inst).__name__
                if tn == "InstLdweights":
                    key = (
                        str(inst.ins[0]),
                        tuple(inst.tile_position or ()),
                        tuple(inst.tile_size or ()),
                        bool(inst.is_transpose),
                    )
                    if key == last_key:
                        nop = mybir.InstNoOp(name=f"ldwnop-{n}", ins=[], outs=[])
                        n += 1
                        nop.engine = inst.engine
                        si = inst.sync_info
                        if si is not None:
                            nop.sync_info = mybir.SyncInfo(
                                on_wait=list(si.on_wait), on_update=list(si.on_update)
                            )
                        out.append(nop)
                        changed = True
                        continue
                    last_key = key
                elif tn == "InstMatmult":
                    if inst.is_transpose:
                        last_key = None
                out.append(inst)
            if changed:
                insts[:] = out
    return n


# ---------------------------------------------------------------------------
# Device program
# ---------------------------------------------------------------------------


def _act_raw(nc, out, in_, func, scale=1.0):
    """Raw InstActivation emitter: bass blocks Rsqrt/Reciprocal on ACT for
    accuracy reasons, but measured on this hardware Rsqrt is accurate to
    4.4e-5 max rel err -- far below this problem's tolerance."""
    eng = nc.scalar
    bias = nc.const_aps.scalar_like(0.0, in_)
    inputs = [eng.lower_ap(in_), eng.lower_ap(bias)]
    for arg in [scale, 0.0]:
        inputs.append(mybir.ImmediateValue(dtype=mybir.dt.float32, value=float(arg)))
    return eng.add_instruction(
        mybir.InstActivation(
            name=nc.get_next_instruction_name(),
            func=func,
            ins=inputs,
            outs=[eng.lower_ap(out)],
        )
    )


def build_program():
    op_dt = dt.float8e4 if USE_FP8 else dt.bfloat16
    descale = 1.0 / (FP8_SCALE * FP8_SCALE) if USE_FP8 else 1.0
    z_sqrt_scale = 1.0 / (2.0 * FP8_SCALE * FP8_SCALE) if USE_FP8 else 0.5
    c_sqrt_scale = (FP8_SCALE * FP8_SCALE) / 2.0 if USE_FP8 else 0.5

    nc = bass.Bass("TRN2", num_devices=NCORES)
    z_in = nc.dram_tensor("z", [ROWS, F, T], dt.float32, kind="ExternalInput")
    c_in = nc.dram_tensor("c", [ROWS, F, T], dt.float32, kind="ExternalInput")
    sims_out = nc.dram_tensor(
        "sims", [ROWS, NBLK, 128, T], dt.float16, kind="ExternalOutput"
    )
    pos_out = nc.dram_tensor("pos", [ROWS, T], dt.float16, kind="ExternalOutput")

    with TileContext(nc) as tc:
        with (
            tc.tile_pool(name="io", bufs=2) as io_pool,
            tc.tile_pool(name="sq", bufs=1) as sq_pool,
            tc.tile_pool(name="rn", bufs=2) as rn_pool,
            tc.tile_pool(name="tmp", bufs=2) as tmp_pool,
            tc.tile_pool(name="scaled", bufs=2) as scaled,
            tc.tile_pool(name="outp", bufs=7) as outp,
            tc.tile_pool(name="gram_ps", bufs=5, space="PSUM") as gram_ps,
            tc.tile_pool(name="stat_ps", bufs=1, space="PSUM") as stat_ps,
        ):
            ones16 = io_pool.tile([128, 128], dt.bfloat16, name="ones16")
            nc.vector.memset(ones16[:], 1.0)

            copy_cols = {"act": 26000.0, "dve": 15000.0}
            COPY_W = {"act": 0.91, "dve": 1.15}

            def drain_psum(dst, src, wc):
                if copy_cols["act"] + COPY_W["act"] * wc <= copy_cols["dve"] + COPY_W["dve"] * wc:
                    eng = "act"
                else:
                    eng = "dve"
                copy_cols[eng] += COPY_W[eng] * wc
                if eng == "act":
                    nc.scalar.activation(
                        dst, src, mybir.ActivationFunctionType.Copy, scale=descale
                    )
                else:
                    nc.vector.tensor_scalar(
                        dst, src, descale, None, mybir.AluOpType.mult
                    )

            state = {}

            def stats_load(r, pieces):
                st = state[r] = {}
                zf = io_pool.tile([128, FCH, T], dt.float32, name="zf", tag="zf")
                cf = io_pool.tile([128, FCH, T], dt.float32, name="cf", tag="cf")
                sqz = sq_pool.tile([128, FCH, T], dt.bfloat16, name="sqz", tag="sqz")
                sqc = sq_pool.tile([128, FCH, T], dt.bfloat16, name="sqc", tag="sqc")
                zc = sq_pool.tile([128, FCH, T], dt.bfloat16, name="zc", tag="zc")
                st.update(zf=zf, cf=cf, sqz=sqz, sqc=sqc, zc=zc)
                pw = T // pieces
                for p in range(pieces):
                    sl = slice(pw * p, pw * (p + 1))
                    for j in range(FCH):
                        nc.sync.dma_start(
                            out=zf[:, j, sl],
                            in_=z_in[r, 128 * j : 128 * (j + 1), sl],
                        )
                        nc.sync.dma_start(
                            out=cf[:, j, sl],
                            in_=c_in[r, 128 * j : 128 * (j + 1), sl],
                        )
                for p in range(pieces):
                    sl = slice(pw * p, pw * (p + 1))
                    for j in range(FCH):
                        nc.scalar.activation(
                            sqz[:, j, sl], zf[:, j, sl],
                            mybir.ActivationFunctionType.Square,
                        )
                        nc.scalar.activation(
                            sqc[:, j, sl], cf[:, j, sl],
                            mybir.ActivationFunctionType.Square,
                        )
                        if USE_GPSIMD:
                            nc.gpsimd.tensor_mul(zc[:, j, sl], zf[:, j, sl], cf[:, j, sl])
                        else:
                            nc.vector.tensor_tensor(
                                out=zc[:, j, sl], in0=zf[:, j, sl],
                                in1=cf[:, j, sl], op=mybir.AluOpType.mult,
                            )
                st["rnz"] = rn_pool.tile([128, T], dt.float32, name="rnz", tag="rnz")
                st["rnc"] = rn_pool.tile([128, T], dt.float32, name="rnc", tag="rnc")
                st["pos16"] = rn_pool.tile([128, T], dt.float16, name="pos16", tag="pos16")

            def stats_norm(r, cc):
                st = state[r]
                sl = slice(512 * cc, 512 * (cc + 1))
                nz_ps = stat_ps.tile([128, 512], dt.float32, name="nz_ps", tag="ps_a")
                ncc_ps = stat_ps.tile([128, 512], dt.float32, name="ncc_ps", tag="ps_b")
                zc_ps = stat_ps.tile([128, 512], dt.float32, name="zc_ps", tag="ps_c")
                for j in range(FCH):
                    b, e = (j == 0), (j == FCH - 1)
                    nc.tensor.matmul(nz_ps[:], ones16[:], st["sqz"][:, j, sl], start=b, stop=e)
                    nc.tensor.matmul(ncc_ps[:], ones16[:], st["sqc"][:, j, sl], start=b, stop=e)
                    nc.tensor.matmul(zc_ps[:], ones16[:], st["zc"][:, j, sl], start=b, stop=e)
                pt = tmp_pool.tile([128, 512], dt.float32, name="pt", tag="pt")
                _act_raw(nc, st["rnz"][:, sl], nz_ps[:],
                         mybir.ActivationFunctionType.Rsqrt, scale=z_sqrt_scale)
                _act_raw(nc, st["rnc"][:, sl], ncc_ps[:],
                         mybir.ActivationFunctionType.Rsqrt, scale=c_sqrt_scale)
                nc.vector.tensor_tensor(
                    out=pt[:], in0=zc_ps[:], in1=st["rnz"][:, sl],
                    op=mybir.AluOpType.mult,
                )
                if USE_GPSIMD:
                    nc.gpsimd.tensor_mul(st["pos16"][:, sl], pt[:], st["rnc"][:, sl])
                else:
                    nc.vector.tensor_tensor(
                        out=st["pos16"][:, sl], in0=pt[:], in1=st["rnc"][:, sl],
                        op=mybir.AluOpType.mult,
                    )

            def stats_zs(r, j, cc=None):
                st = state[r]
                if "zs" not in st:
                    st["zs"] = scaled.tile([128, FCH, T], op_dt, name="zs", tag="zs")
                sl = slice(0, T) if cc is None else slice(512 * cc, 512 * (cc + 1))
                nc.vector.tensor_tensor(
                    out=st["zs"][:, j, sl], in0=st["zf"][:, j, sl],
                    in1=st["rnz"][:, sl], op=mybir.AluOpType.mult,
                )

            def stats_posdma(r):
                nc.sync.dma_start(out=pos_out[r, :], in_=state[r]["pos16"][0:1, :])

            def emit_gram_chunk(r, tau, off, wc, odst):
                zs = state[r]["zs"]
                t0 = 128 * tau
                ps = gram_ps.tile([128, 512], dt.float32, name="ps", tag="ps")
                if USE_FP8:
                    nc.tensor.matmul(
                        ps[:, :wc], zs[:, :, t0 : t0 + 128],
                        zs[:, :, t0 + off : t0 + off + wc],
                        start=True, stop=True,
                        perf_mode=mybir.MatmulPerfMode.DoubleRow,
                    )
                else:
                    for j in range(FCH):
                        nc.tensor.matmul(
                            ps[:, :wc], zs[:, j, t0 : t0 + 128],
                            zs[:, j, t0 + off : t0 + off + wc],
                            start=(j == 0), stop=(j == FCH - 1),
                        )
                drain_psum(odst[:, off : off + wc], ps[:, :wc], wc)

            def emit_gram_block(r, tau):
                w = T - 128 * tau
                otile = outp.tile([128, T], dt.float16, name="otile", tag="otile")
                off = 0
                while off < w:
                    wc = min(512, w - off)
                    emit_gram_chunk(r, tau, off, wc, otile)
                    off += wc
                nc.sync.dma_start(out=sims_out[r, tau, :, :w], in_=otile[:, :w])

            sid = nc.enter_named_scope("stats_r0", False)[0]
            stats_load(0, pieces=4)
            for cc in range(4):
                stats_norm(0, cc)
                for j in range(FCH):
                    stats_zs(0, j, cc)
            stats_posdma(0)
            nc.leave_named_scope("stats_r0", sid, False)

            for r in range(ROWS):
                nxt = r + 1
                phases = {}
                if nxt < ROWS:
                    phases = {
                        1: lambda: stats_load(nxt, pieces=1),
                        3: lambda: stats_norm(nxt, 0),
                        4: lambda: stats_norm(nxt, 1),
                        5: lambda: stats_norm(nxt, 2),
                        6: lambda: stats_norm(nxt, 3),
                        7: lambda: stats_zs(nxt, 0),
                        8: lambda: stats_zs(nxt, 1),
                        9: lambda: stats_posdma(nxt),
                    }
                sid = nc.enter_named_scope(f"gram_r{r}", False)[0]
                for tau in range(NBLK):
                    if tau in phases:
                        phases[tau]()
                    emit_gram_block(r, tau)
                nc.leave_named_scope(f"gram_r{r}", sid, False)

    dedup_ldweights(nc)
    split_excess_waits(nc)
    return nc


_PROGRAM = None


def _get_program():
    global _PROGRAM
    if _PROGRAM is None:
        _PROGRAM = build_program()
    return _PROGRAM


def kernel(z, c, negative_inds, _trace=False):
    z = np.ascontiguousarray(np.asarray(z, dtype=np.float32))
    c = np.ascontiguousarray(np.asarray(c, dtype=np.float32))
    ni = np.asarray(negative_inds)
    assert z.shape == (B, F, T) and c.shape == (B, F, T + 1)

    c_sl = np.ascontiguousarray(c[:, :, 1:])  # [B, F, T]

    nc = _get_program()
    in_maps = []
    for core in range(NCORES):
        rs = slice(core * ROWS, (core + 1) * ROWS)
        in_maps.append({"z": z[rs], "c": c_sl[rs]})

    res = run_bass_kernel_spmd(nc, in_maps, list(range(NCORES)), trace=_trace)

    sims = np.concatenate(
        [res.results[i]["sims"] for i in range(NCORES)], axis=0
    )
    pos = np.concatenate(
        [res.results[i]["pos"] for i in range(NCORES)], axis=0
    )

    n = ni.reshape(B, T, K).astype(np.int64)
    t = np.arange(T, dtype=np.int64)[None, :, None]
    swap = (n >> 7) < (t >> 7)
    a = np.where(swap, n, t)
    bcol = np.where(swap, t, n)
    bidx = np.arange(B)[:, None, None]
    neg = sims[bidx, a >> 7, a & 127, bcol - ((a >> 7) << 7)]
    logits = np.concatenate([pos[:, :, None], neg], axis=2).astype(np.float32)
    out = logits.reshape(B * T, K + 1)
    if _trace:
        return out, res
    return out


if __name__ == "__main__":
    rng = np.random.default_rng(0)
    z = rng.standard_normal((B, F, T), dtype=np.float32)
    c = rng.standard_normal((B, F, T + 1), dtype=np.float32)
    ni = rng.integers(0, T - 1, size=(B, T * K)).astype(np.int64)
    out = kernel(z=z, c=c, negative_inds=ni)
    print("out", out.shape, out.dtype, np.isfinite(out).all())
